# revision 2
# baseline (speedup 1.0000x reference)
"""Bayesian SkipGram forward pass on 8 Trainium2 cores.

Strategy (vocab/model parallel, per the V-axis sharding):
  - V=50000 is split into 8 shards of 6250, each padded to 6272 = 49*128.
  - Each core holds its shard of E (transposed), prior_sigma (transposed),
    W_gen (transposed) and b_gen, plus replicated copies of the tiny
    Z/2D-sized tensors.
  - Phase A (per core): partial one-hot matvecs  pce = E_shard @ center,
    pcw = E_shard @ ctx_onehots, pzsig = prior_sigma_shard @ center via the
    tensor engine (contraction over the V shard).  One small AllGather
    combines the 8 partial vectors; every core reduces them locally.
  - Replicated MLP: relu/sum -> u, s (softplus), z = u + eps*s, and the KL
    terms (all [128]-sized, cheap).
  - Phase B: logits_shard = W_gen_shard @ z + b_gen_shard; local max and
    local sum(exp(logit - local_max)); a second tiny AllGather of the
    (max_m, sumexp_m) pairs lets every core form the exact global
    log_softmax denominator without two AllReduce round-trips.
  - loss_probs gather: logits at context_word_idxs are recomputed exactly
    from host-gathered rows W_gen[idxs, :] (index gather, done once on the
    host) so no cross-shard index traffic is needed.
  - prior_mean is unused by the reference model and is never transferred.

The final scalar is computed redundantly on every core; core 0's output is
returned.
"""

import os
import sys
import types

import numpy as np


def _install_ntff_hook():
    """Fail-soft shim: the agent image's antenv lacks axon_hooks, which
    bass_utils imports when tracing is requested (BASS_TRACE=1)."""
    try:
        if "antenv.axon_hooks" in sys.modules:
            return
        import antenv

        mod = types.ModuleType("antenv.axon_hooks")
        mod._hook = None

        def set_axon_ntff_profile_hook(h):
            mod._hook = h

        def get_axon_ntff_profile_hook():
            return mod._hook

        mod.set_axon_ntff_profile_hook = set_axon_ntff_profile_hook
        mod.get_axon_ntff_profile_hook = get_axon_ntff_profile_hook
        sys.modules["antenv.axon_hooks"] = mod
        antenv.axon_hooks = mod
        try:
            from trn_agent_boot.trn_boot import _ntff_profile_via_ctypes

            set_axon_ntff_profile_hook(
                _ntff_profile_via_ctypes("/opt/axon/libaxon_pjrt.so")
            )
        except Exception:
            pass
    except Exception:
        pass


_install_ntff_hook()

import concourse.bacc as bacc
import concourse.mybir as mybir
import concourse.tile as tile
from concourse.bass_utils import run_bass_kernel_spmd

V, D, Z, C = 50000, 300, 128, 10
M = 8  # cores
VS = V // M  # 6250 real elements per shard
T = 49  # 128-wide v-tiles per shard
VP = T * 128  # 6272 padded shard size
ETG = 7  # E tiles (7 v-tiles each) for DMA/compute overlap
PWG = 7  # prior_sigma / W_gen tile split
F32 = mybir.dt.float32
AF = mybir.ActivationFunctionType
NEG = -1.0e30
# phase-A D chunking: 300 = 128 + 128 + 44
DCH = [(0, 128), (128, 128), (256, 44)]
PACKW = 34  # 3 chunks * 11 columns + 1 zsig column


def _shard_inputs(inputs):
    """Host-side: slice/pad/transpose the full tensors into per-core device
    layouts.  Returns list of 8 in_maps."""
    E = np.asarray(inputs["E"], np.float32)
    psig = np.asarray(inputs["prior_sigma"], np.float32)
    wgen = np.asarray(inputs["W_gen"], np.float32)
    bgen = np.asarray(inputs["b_gen"], np.float32)
    center = np.asarray(inputs["center_word"], np.float32)
    ctx = np.asarray(inputs["context_words"], np.float32)
    idxs = np.asarray(inputs["context_word_idxs"]).astype(np.int64)

    # replicated smalls
    wmu = np.asarray(inputs["W_mu"], np.float32)
    wsig = np.asarray(inputs["W_sig"], np.float32)

    def pad_mlp(w):  # [Z, 600] -> [128, 768] col-padded per 384-half
        out = np.zeros((Z, 768), np.float32)
        out[:, 0:300] = w[:, 0:300]
        out[:, 384:684] = w[:, 300:600]
        # -> wmt[p, j*128+z] = out[z, j*128+p]
        return np.ascontiguousarray(
            out.T.reshape(6, 128, Z).transpose(1, 0, 2).reshape(128, 768)
        )

    wmt = pad_mlp(wmu)
    wst = pad_mlp(wsig)
    bmu = np.ascontiguousarray(np.asarray(inputs["b_mu"], np.float32))
    bsg = np.ascontiguousarray(np.asarray(inputs["b_sig"], np.float32))
    eps = np.ascontiguousarray(np.asarray(inputs["eps"], np.float32))
    wgc = np.ascontiguousarray(wgen[idxs, :].T)  # [Z, C]
    bgc = np.ascontiguousarray(bgen[idxs])  # [C]
    idt = np.eye(128, dtype=np.float32)

    maps = []
    for m in range(M):
        lo = m * VS
        hi = lo + VS
        # E shard -> et[p, t*300+d] = E[d, lo + t*128 + p]
        e = np.zeros((D, VP), np.float32)
        e[:, :VS] = E[:, lo:hi]
        et = np.ascontiguousarray(
            e.reshape(D, T, 128).transpose(2, 1, 0).reshape(128, T * D)
        )
        # one-hots -> oh[p, t*11+0]=center, [p, t*11+1+c]=ctx[c]
        cw = np.zeros((VP,), np.float32)
        cw[:VS] = center[lo:hi]
        xw = np.zeros((C, VP), np.float32)
        xw[:, :VS] = ctx[:, lo:hi]
        oh = np.concatenate(
            [
                cw.reshape(T, 128).T[:, :, None],  # [128, T, 1]
                xw.reshape(C, T, 128).transpose(2, 1, 0),  # [128, T, C]
            ],
            axis=2,
        ).reshape(128, T * (C + 1))
        oh = np.ascontiguousarray(oh)
        # prior_sigma shard -> pst[p, t*128+z] = psig[z, lo + t*128 + p]
        p = np.zeros((Z, VP), np.float32)
        p[:, :VS] = psig[:, lo:hi]
        pst = np.ascontiguousarray(
            p.reshape(Z, T, 128).transpose(2, 1, 0).reshape(128, T * Z)
        )
        # W_gen shard -> wgt[z, t*128+p] = wgen[lo + t*128 + p, z]
        w = np.zeros((VP, Z), np.float32)
        w[:VS, :] = wgen[lo:hi, :]
        wgt = np.ascontiguousarray(
            w.reshape(T, 128, Z).transpose(2, 0, 1).reshape(Z, T * 128)
        )
        # b_gen shard -> bgt[p, t]; padding gets a huge negative bias so the
        # pad logits can never win the max and exp() maps them to zero.
        b = np.full((VP,), NEG, np.float32)
        b[:VS] = bgen[lo:hi]
        bgt = np.ascontiguousarray(b.reshape(T, 128).T)

        maps.append(
            {
                "et": et,
                "oh": oh,
                "pst": pst,
                "wgt": wgt,
                "bgt": bgt,
                "wmt": wmt,
                "wst": wst,
                "bmu": bmu,
                "bsg": bsg,
                "eps": eps,
                "wgc": wgc,
                "bgc": bgc,
                "idt": idt,
            }
        )
    return maps


def _build():
    nc = bacc.Bacc("TRN2", target_bir_lowering=False, debug=False, num_devices=M)

    et_d = nc.dram_tensor("et", [128, T * D], F32, kind="ExternalInput")
    oh_d = nc.dram_tensor("oh", [128, T * (C + 1)], F32, kind="ExternalInput")
    pst_d = nc.dram_tensor("pst", [128, T * Z], F32, kind="ExternalInput")
    wgt_d = nc.dram_tensor("wgt", [128, T * 128], F32, kind="ExternalInput")
    bgt_d = nc.dram_tensor("bgt", [128, T], F32, kind="ExternalInput")
    wmt_d = nc.dram_tensor("wmt", [128, 768], F32, kind="ExternalInput")
    wst_d = nc.dram_tensor("wst", [128, 768], F32, kind="ExternalInput")
    bmu_d = nc.dram_tensor("bmu", [Z], F32, kind="ExternalInput")
    bsg_d = nc.dram_tensor("bsg", [Z], F32, kind="ExternalInput")
    eps_d = nc.dram_tensor("eps", [Z], F32, kind="ExternalInput")
    wgc_d = nc.dram_tensor("wgc", [Z, C], F32, kind="ExternalInput")
    bgc_d = nc.dram_tensor("bgc", [C], F32, kind="ExternalInput")
    idt_d = nc.dram_tensor("idt", [128, 128], F32, kind="ExternalInput")
    out_d = nc.dram_tensor("out", [1], F32, kind="ExternalOutput")

    ecols = T // ETG * D  # 2100
    pcols = T // PWG * Z  # 896
    rg = [list(range(M))]

    with tile.TileContext(nc) as tc:
        with (
            tc.tile_pool(name="sb", bufs=1) as sb,
            tc.tile_pool(name="ps", bufs=1, space="PSUM") as ps,
            tc.tile_pool(name="dram", bufs=1, space="DRAM") as dram,
        ):
            # ---- input DMAs (program order ~ priority) ----
            oh_sb = sb.tile([128, T * (C + 1)], F32)
            nc.sync.dma_start(oh_sb[:], oh_d[:])
            et_sb = []
            for g in range(ETG):
                t_ = sb.tile([128, ecols], F32, name=f"et{g}", tag=f"et{g}")
                nc.sync.dma_start(
                    t_[:], et_d[:, g * ecols : (g + 1) * ecols]
                )
                et_sb.append(t_)
            pst_sb = []
            for g in range(PWG):
                t_ = sb.tile([128, pcols], F32, name=f"pst{g}", tag=f"pst{g}")
                nc.sync.dma_start(
                    t_[:], pst_d[:, g * pcols : (g + 1) * pcols]
                )
                pst_sb.append(t_)
            wmt_sb = sb.tile([128, 768], F32)
            nc.sync.dma_start(wmt_sb[:], wmt_d[:])
            wst_sb = sb.tile([128, 768], F32)
            nc.sync.dma_start(wst_sb[:], wst_d[:])
            bmu_sb = sb.tile([Z, 1], F32)
            nc.sync.dma_start(bmu_sb[:], bmu_d[:])
            bsg_sb = sb.tile([Z, 1], F32)
            nc.sync.dma_start(bsg_sb[:], bsg_d[:])
            eps_sb = sb.tile([Z, 1], F32)
            nc.sync.dma_start(eps_sb[:], eps_d[:])
            wgc_sb = sb.tile([Z, C], F32)
            nc.sync.dma_start(wgc_sb[:], wgc_d[:])
            bgc_sb = sb.tile([C, 1], F32)
            nc.sync.dma_start(bgc_sb[:], bgc_d[:])
            idt_sb = sb.tile([128, 128], F32)
            nc.sync.dma_start(idt_sb[:], idt_d[:])
            bgt_sb = sb.tile([128, T], F32)
            nc.sync.dma_start(bgt_sb[:], bgt_d[:])
            wgt_sb = []
            for g in range(PWG):
                t_ = sb.tile([128, pcols], F32, name=f"wgt{g}", tag=f"wgt{g}")
                nc.sync.dma_start(
                    t_[:], wgt_d[:, g * pcols : (g + 1) * pcols]
                )
                wgt_sb.append(t_)

            ones_sb = sb.tile([128, 1], F32)
            nc.vector.memset(ones_sb[:], 1.0)
            onesr_sb = sb.tile([1, 128], F32)
            nc.vector.memset(onesr_sb[:], 1.0)

            # ---- phase A: partial E / prior_sigma contractions ----
            # pA columns: [0:11] d-chunk0, [11:22] d-chunk1, [22:33] d-chunk2
            # (rows 0:44 valid), [33:34] zsig partial.
            pA = ps.tile([128, PACKW], F32)
            for t in range(T):
                g, r = divmod(t, T // ETG)
                rhs = oh_sb[:, t * (C + 1) : (t + 1) * (C + 1)]
                for j, (dlo, dw) in enumerate(DCH):
                    nc.tensor.matmul(
                        pA[0:dw, j * 11 : (j + 1) * 11],
                        et_sb[g][:, r * D + dlo : r * D + dlo + dw],
                        rhs,
                        start=(t == 0),
                        stop=(t == T - 1),
                        skip_group_check=True,
                    )
            for t in range(T):
                g, r = divmod(t, T // PWG)
                nc.tensor.matmul(
                    pA[:, 33:34],
                    pst_sb[g][:, r * Z : (r + 1) * Z],
                    oh_sb[:, t * (C + 1) : t * (C + 1) + 1],
                    start=(t == 0),
                    stop=(t == T - 1),
                    skip_group_check=True,
                )

            pack = sb.tile([128, PACKW], F32)
            nc.vector.memset(pack[:], 0.0)
            nc.vector.tensor_copy(pack[:, 0:22], pA[:, 0:22])
            nc.vector.tensor_copy(pack[0:44, 22:33], pA[0:44, 22:33])
            nc.vector.tensor_copy(pack[:, 33:34], pA[:, 33:34])

            ag1_in = dram.tile([128, PACKW], F32)
            ag1_out = dram.tile([M, 128, PACKW], F32, addr_space="Shared")
            nc.sync.dma_start(ag1_in[:], pack[:])
            nc.gpsimd.collective_compute(
                "AllGather",
                mybir.AluOpType.bypass,
                ins=[ag1_in.opt()],
                outs=[ag1_out.opt()],
                replica_groups=rg,
            )
            agg_sb = sb.tile([128, M, PACKW], F32)
            nc.sync.dma_start(agg_sb[:], ag1_out[:].rearrange("r p f -> p r f"))
            S = sb.tile([128, PACKW], F32)
            nc.vector.reduce_sum(
                S[:], agg_sb[:].rearrange("p r f -> p f r"), axis=mybir.AxisListType.X
            )

            # ---- replicated MLP ----
            R = sb.tile([128, 33], F32)
            nc.scalar.activation(R[:], S[:, 0:33], AF.Relu)
            s_all = sb.tile([128, 6], F32)
            for j in range(3):
                nc.vector.tensor_scalar_mul(
                    s_all[:, j : j + 1], R[:, j * 11 : j * 11 + 1], float(C)
                )
                nc.vector.reduce_sum(
                    s_all[:, 3 + j : 4 + j],
                    R[:, j * 11 + 1 : (j + 1) * 11],
                    axis=mybir.AxisListType.X,
                )
            p_u = ps.tile([Z, 1], F32)
            p_s = ps.tile([Z, 1], F32)
            for j in range(6):
                nc.tensor.matmul(
                    p_u[:],
                    wmt_sb[:, j * 128 : (j + 1) * 128],
                    s_all[:, j : j + 1],
                    start=(j == 0),
                    stop=(j == 5),
                    skip_group_check=True,
                )
                nc.tensor.matmul(
                    p_s[:],
                    wst_sb[:, j * 128 : (j + 1) * 128],
                    s_all[:, j : j + 1],
                    start=(j == 0),
                    stop=(j == 5),
                    skip_group_check=True,
                )
            u_sb = sb.tile([Z, 1], F32)
            nc.vector.tensor_tensor(
                u_sb[:], p_u[:], bmu_sb[:], op=mybir.AluOpType.add
            )
            spre = sb.tile([Z, 1], F32)
            nc.vector.tensor_tensor(
                spre[:], p_s[:], bsg_sb[:], op=mybir.AluOpType.add
            )

            def softplus(dst, src):
                # numerically stable: relu(x) + ln(1 + exp(-|x|))
                a = sb.tile([Z, 1], F32, name=f"sp_a{dst.name}", tag="sp_a")
                nc.scalar.activation(a[:], src, AF.Relu)
                na = sb.tile([Z, 1], F32, name=f"sp_n{dst.name}", tag="sp_n")
                nc.scalar.activation(na[:], src, AF.Abs)
                e = sb.tile([Z, 1], F32, name=f"sp_e{dst.name}", tag="sp_e")
                nc.scalar.activation(e[:], na[:], AF.Exp, scale=-1.0)
                nc.vector.tensor_scalar_add(e[:], e[:], 1.0)
                l = sb.tile([Z, 1], F32, name=f"sp_l{dst.name}", tag="sp_l")
                nc.scalar.activation(l[:], e[:], AF.Ln)
                nc.vector.tensor_tensor(dst, a[:], l[:], op=mybir.AluOpType.add)

            s_sb = sb.tile([Z, 1], F32)
            softplus(s_sb[:], spre[:])
            zs_sb = sb.tile([Z, 1], F32)
            softplus(zs_sb[:], S[:, 33:34])
            z_sb = sb.tile([Z, 1], F32)
            nc.vector.tensor_tensor(
                z_sb[:], eps_sb[:], s_sb[:], op=mybir.AluOpType.mult
            )
            nc.vector.tensor_tensor(
                z_sb[:], z_sb[:], u_sb[:], op=mybir.AluOpType.add
            )

            # KL pieces: ln(zs) - ln(s) + (s^2 + (u-zs)^2)/(2 zs^2) - 0.5
            kl = sb.tile([Z, 1], F32)
            t1 = sb.tile([Z, 1], F32)
            t2 = sb.tile([Z, 1], F32)
            nc.scalar.activation(kl[:], zs_sb[:], AF.Ln)
            nc.scalar.activation(t1[:], s_sb[:], AF.Ln)
            nc.vector.tensor_tensor(kl[:], kl[:], t1[:], op=mybir.AluOpType.subtract)
            nc.vector.tensor_tensor(
                t1[:], u_sb[:], zs_sb[:], op=mybir.AluOpType.subtract
            )
            nc.vector.tensor_tensor(t1[:], t1[:], t1[:], op=mybir.AluOpType.mult)
            nc.vector.tensor_tensor(t2[:], s_sb[:], s_sb[:], op=mybir.AluOpType.mult)
            nc.vector.tensor_tensor(t1[:], t1[:], t2[:], op=mybir.AluOpType.add)
            nc.vector.reciprocal(t2[:], zs_sb[:])
            nc.vector.tensor_tensor(t2[:], t2[:], t2[:], op=mybir.AluOpType.mult)
            nc.vector.tensor_tensor(t1[:], t1[:], t2[:], op=mybir.AluOpType.mult)
            nc.vector.tensor_scalar_mul(t1[:], t1[:], 0.5)
            nc.vector.tensor_tensor(kl[:], kl[:], t1[:], op=mybir.AluOpType.add)
            nc.vector.tensor_scalar_add(kl[:], kl[:], -0.5)
            p_kl = ps.tile([1, 1], F32, tag="tiny")
            nc.tensor.matmul(
                p_kl[:], kl[:], ones_sb[:], start=True, stop=True
            )
            klsum = sb.tile([1, 1], F32)
            nc.vector.tensor_copy(klsum[:], p_kl[:])

            # ---- phase B: logits shard ----
            p_l = ps.tile([128, T], F32)
            for t in range(T):
                g, r = divmod(t, T // PWG)
                nc.tensor.matmul(
                    p_l[:, t : t + 1],
                    wgt_sb[g][:, r * 128 : (r + 1) * 128],
                    z_sb[:],
                    start=True,
                    stop=True,
                    skip_group_check=True,
                )
            logits = sb.tile([128, T], F32)
            nc.vector.tensor_tensor(
                logits[:], p_l[:], bgt_sb[:], op=mybir.AluOpType.add
            )
            lmaxp = sb.tile([128, 1], F32)
            nc.vector.reduce_max(lmaxp[:], logits[:], axis=mybir.AxisListType.X)
            p_t = ps.tile([1, 128], F32, tag="tiny2")
            nc.tensor.transpose(p_t[:], lmaxp[:], idt_sb[:])
            lmaxr = sb.tile([1, 128], F32)
            nc.vector.tensor_copy(lmaxr[:], p_t[:])
            lmax = sb.tile([1, 1], F32)
            nc.vector.reduce_max(lmax[:], lmaxr[:], axis=mybir.AxisListType.X)
            nlmax = sb.tile([1, 1], F32)
            nc.vector.tensor_scalar_mul(nlmax[:], lmax[:], -1.0)
            p_b = ps.tile([128, 1], F32, tag="tiny")
            nc.tensor.matmul(
                p_b[:], onesr_sb[:], nlmax[:], start=True, stop=True
            )
            nlb = sb.tile([128, 1], F32)
            nc.vector.tensor_copy(nlb[:], p_b[:])
            ex = sb.tile([128, T], F32)
            esum = sb.tile([128, 1], F32)
            nc.scalar.activation(
                ex[:], logits[:], AF.Exp, bias=nlb[:], accum_out=esum[:]
            )
            p_e = ps.tile([1, 1], F32, tag="tiny2")
            nc.tensor.matmul(
                p_e[:], esum[:], ones_sb[:], start=True, stop=True
            )

            pair = sb.tile([1, 8], F32)
            nc.vector.memset(pair[:], 0.0)
            nc.vector.tensor_copy(pair[:, 0:1], lmax[:])
            nc.vector.tensor_copy(pair[:, 1:2], p_e[:])
            ag2_in = dram.tile([8], F32)
            ag2_out = dram.tile([M, 8], F32, addr_space="Shared")
            nc.sync.dma_start(ag2_in[:], pair[:])
            nc.gpsimd.collective_compute(
                "AllGather",
                mybir.AluOpType.bypass,
                ins=[ag2_in.opt()],
                outs=[ag2_out.opt()],
                replica_groups=rg,
            )
            agp = sb.tile([1, M * 8], F32)
            nc.sync.dma_start(agp[:], ag2_out[:].rearrange("r f -> (r f)"))

            gmax = sb.tile([1, 1], F32)
            nc.vector.reduce_max(
                gmax[:], agp[:, 0 : M * 8 : 8], axis=mybir.AxisListType.X
            )
            w8 = sb.tile([1, M], F32)
            nc.vector.tensor_scalar(
                w8[:],
                agp[:, 0 : M * 8 : 8],
                gmax[:],
                None,
                op0=mybir.AluOpType.subtract,
            )
            nc.scalar.activation(w8[:], w8[:], AF.Exp)
            nc.vector.tensor_tensor(
                w8[:], w8[:], agp[:, 1 : M * 8 : 8], op=mybir.AluOpType.mult
            )
            gsum = sb.tile([1, 1], F32)
            nc.vector.reduce_sum(gsum[:], w8[:], axis=mybir.AxisListType.X)

            # ---- context logits from host-gathered W_gen rows ----
            p_c = ps.tile([C, 1], F32, tag="tiny")
            nc.tensor.matmul(p_c[:], wgc_sb[:], z_sb[:], start=True, stop=True)
            cl = sb.tile([C, 1], F32)
            nc.vector.tensor_tensor(
                cl[:], p_c[:], bgc_sb[:], op=mybir.AluOpType.add
            )
            p_cs = ps.tile([1, 1], F32, tag="tiny2")
            nc.tensor.matmul(
                p_cs[:], cl[:], ones_sb[0:C, :], start=True, stop=True
            )

            # ---- final scalar ----
            res = sb.tile([1, 1], F32)
            nc.scalar.activation(res[:], gsum[:], AF.Ln)
            nc.vector.tensor_tensor(
                res[:], res[:], gmax[:], op=mybir.AluOpType.add
            )
            nc.vector.tensor_scalar_mul(res[:], res[:], float(C))
            tfin = sb.tile([1, 1], F32)
            nc.vector.tensor_tensor(
                tfin[:], p_cs[:], res[:], op=mybir.AluOpType.subtract
            )
            nc.vector.tensor_tensor(
                tfin[:], tfin[:], klsum[:], op=mybir.AluOpType.subtract
            )
            nc.sync.dma_start(out_d[:], tfin[:])

    nc.compile()
    return nc


_NC = None


def kernel(**inputs) -> np.ndarray:
    global _NC
    if _NC is None:
        _NC = _build()
    in_maps = _shard_inputs(inputs)
    res = run_bass_kernel_spmd(
        _NC,
        in_maps,
        list(range(M)),
        trace=bool(os.environ.get("KERNEL_TRACE")),
    )
    out = np.float32(res.results[0]["out"][0])
    kernel.last_exec_time_ns = res.exec_time_ns
    kernel.last_profile_json = res.profile_json
    return np.asarray(out, dtype=np.float32).reshape(())


def emulate(**inputs) -> np.ndarray:
    """Numpy emulation of the exact device dataflow (for layout validation)."""
    maps = _shard_inputs(inputs)
    packs = []
    for m in range(M):
        mp = maps[m]
        et, oh, pst = mp["et"], mp["oh"], mp["pst"]
        pack = np.zeros((128, PACKW), np.float32)
        for t in range(T):
            rhs = oh[:, t * (C + 1) : (t + 1) * (C + 1)]  # [128, 11]
            for j, (dlo, dw) in enumerate(DCH):
                lhsT = et[:, t * D + dlo : t * D + dlo + dw]  # [128, dw]
                pack[0:dw, j * 11 : (j + 1) * 11] += lhsT.T @ rhs
            lhsT = pst[:, t * Z : (t + 1) * Z]
            pack[:, 33] += lhsT.T @ oh[:, t * (C + 1)]
        packs.append(pack)
    S = np.sum(packs, axis=0)  # AllGather + local reduce
    R = np.maximum(S[:, 0:33], 0.0)
    s_all = np.zeros((128, 6), np.float32)
    for j in range(3):
        s_all[:, j] = C * R[:, j * 11]
        s_all[:, 3 + j] = R[:, j * 11 + 1 : (j + 1) * 11].sum(axis=1)
    mp = maps[0]

    def mlp(wmt):
        acc = np.zeros((Z,), np.float32)
        for j in range(6):
            acc += wmt[:, j * 128 : (j + 1) * 128].T @ s_all[:, j]
        return acc

    def sp(x):
        return np.maximum(x, 0) + np.log1p(np.exp(-np.abs(x)))

    u = mlp(mp["wmt"]) + mp["bmu"]
    s = sp(mlp(mp["wst"]) + mp["bsg"])
    z = u + mp["eps"] * s
    zs = sp(S[:, 33])
    kl = np.log(zs) - np.log(s) + (s**2 + (u - zs) ** 2) * 0.5 / zs**2 - 0.5
    klsum = kl.sum()

    pairs = []
    for m in range(M):
        mp_ = maps[m]
        logits = np.zeros((128, T), np.float32)
        for t in range(T):
            logits[:, t] = mp_["wgt"][:, t * 128 : (t + 1) * 128].T @ z
        logits += mp_["bgt"]
        lmax = logits.max()
        esum = np.exp(logits - lmax).sum()
        pairs.append((lmax, esum))
    gmax = max(p[0] for p in pairs)
    gsum = sum(p[1] * np.exp(p[0] - gmax) for p in pairs)
    cl = mp["wgc"].T @ z + mp["bgc"]
    resv = cl.sum() - C * (gmax + np.log(gsum)) - klsum
    return np.asarray(np.float32(resv)).reshape(())


# revision 3
# speedup vs baseline: 1.4512x; 1.4512x over previous
"""Bayesian SkipGram forward pass on 8 Trainium2 cores.

Strategy (vocab/model parallel, per the V-axis sharding):
  - V=50000 is split into 8 shards of 6250, each padded to 6272 = 49*128.
  - Each core holds its shard of E (transposed), prior_sigma (transposed),
    W_gen (transposed) and b_gen, plus replicated copies of the tiny
    Z/2D-sized tensors.
  - Phase A (per core): partial one-hot matvecs  pce = E_shard @ center,
    pcw = E_shard @ ctx_onehots, pzsig = prior_sigma_shard @ center via the
    tensor engine (contraction over the V shard).  One small AllGather
    combines the 8 partial vectors; every core reduces them locally.
  - Replicated MLP: relu/sum -> u, s (softplus), z = u + eps*s, and the KL
    terms (all [128]-sized, cheap).
  - Phase B: logits_shard = W_gen_shard @ z + b_gen_shard; local max and
    local sum(exp(logit - local_max)); a second tiny AllGather of the
    (max_m, sumexp_m) pairs lets every core form the exact global
    log_softmax denominator without two AllReduce round-trips.
  - loss_probs gather: logits at context_word_idxs are recomputed exactly
    from host-gathered rows W_gen[idxs, :] (index gather, done once on the
    host) so no cross-shard index traffic is needed.
  - prior_mean is unused by the reference model and is never transferred.
  - A dummy AllGather issued at kernel start absorbs the collective
    communicator bootstrap concurrently with the input DMA phase.

The final scalar is computed redundantly on every core; core 0's output is
returned.  Inputs are pre-staged onto the 8 devices (device_put + block)
before the NEFF executes so all ranks start aligned.
"""

import glob
import os
import sys
import tempfile
import types

import numpy as np


def _install_ntff_hook():
    """Fail-soft shim: the agent image's antenv lacks axon_hooks, which
    bass_utils imports when tracing is requested."""
    try:
        if "antenv.axon_hooks" in sys.modules:
            return
        import antenv

        mod = types.ModuleType("antenv.axon_hooks")
        mod._hook = None

        def set_axon_ntff_profile_hook(h):
            mod._hook = h

        def get_axon_ntff_profile_hook():
            return mod._hook

        mod.set_axon_ntff_profile_hook = set_axon_ntff_profile_hook
        mod.get_axon_ntff_profile_hook = get_axon_ntff_profile_hook
        sys.modules["antenv.axon_hooks"] = mod
        antenv.axon_hooks = mod
        try:
            from trn_agent_boot.trn_boot import _ntff_profile_via_ctypes

            set_axon_ntff_profile_hook(
                _ntff_profile_via_ctypes("/opt/axon/libaxon_pjrt.so")
            )
        except Exception:
            pass
    except Exception:
        pass


_install_ntff_hook()

import concourse.bacc as bacc
import concourse.bass_utils as bass_utils
import concourse.mybir as mybir
import concourse.tile as tile

V, D, Z, C = 50000, 300, 128, 10
M = 8  # cores
VS = V // M  # 6250 real elements per shard
T = 49  # 128-wide v-tiles per shard
VP = T * 128  # 6272 padded shard size
ETG = 7  # E tiles (7 v-tiles each) for DMA/compute overlap
PWG = 7  # prior_sigma / W_gen tile split
F32 = mybir.dt.float32
AF = mybir.ActivationFunctionType
ALU = mybir.AluOpType
NEG = -1.0e30
# phase-A D chunking: 300 = 128 + 128 + 44
DCH = [(0, 128), (128, 128), (256, 44)]
PACKW = 34  # 3 chunks * 11 columns + 1 zsig column
WARMUP_CC = True


def _shard_inputs(inputs):
    """Host-side: slice/pad/transpose the full tensors into per-core device
    layouts.  Returns list of 8 in_maps."""
    E = np.asarray(inputs["E"], np.float32)
    psig = np.asarray(inputs["prior_sigma"], np.float32)
    wgen = np.asarray(inputs["W_gen"], np.float32)
    bgen = np.asarray(inputs["b_gen"], np.float32)
    center = np.asarray(inputs["center_word"], np.float32)
    ctx = np.asarray(inputs["context_words"], np.float32)
    idxs = np.asarray(inputs["context_word_idxs"]).astype(np.int64)

    wmu = np.asarray(inputs["W_mu"], np.float32)
    wsig = np.asarray(inputs["W_sig"], np.float32)

    def pad_mlp(w):  # [Z, 600] -> [128, 768] col-padded per 384-half
        out = np.zeros((Z, 768), np.float32)
        out[:, 0:300] = w[:, 0:300]
        out[:, 384:684] = w[:, 300:600]
        return np.ascontiguousarray(
            out.T.reshape(6, 128, Z).transpose(1, 0, 2).reshape(128, 768)
        )

    wmt = pad_mlp(wmu)
    wst = pad_mlp(wsig)
    bmu = np.ascontiguousarray(np.asarray(inputs["b_mu"], np.float32))
    bsg = np.ascontiguousarray(np.asarray(inputs["b_sig"], np.float32))
    eps = np.ascontiguousarray(np.asarray(inputs["eps"], np.float32))
    wgc = np.ascontiguousarray(wgen[idxs, :].T)  # [Z, C]
    bgc = np.ascontiguousarray(bgen[idxs])  # [C]
    idt = np.eye(128, dtype=np.float32)

    maps = []
    for m in range(M):
        lo = m * VS
        hi = lo + VS
        # E shard -> et[p, t*300+d] = E[d, lo + t*128 + p]
        e = np.zeros((D, VP), np.float32)
        e[:, :VS] = E[:, lo:hi]
        et = np.ascontiguousarray(
            e.reshape(D, T, 128).transpose(2, 1, 0).reshape(128, T * D)
        )
        # one-hots -> oh[p, t*11+0]=center, [p, t*11+1+c]=ctx[c]
        cw = np.zeros((VP,), np.float32)
        cw[:VS] = center[lo:hi]
        xw = np.zeros((C, VP), np.float32)
        xw[:, :VS] = ctx[:, lo:hi]
        oh = np.concatenate(
            [
                cw.reshape(T, 128).T[:, :, None],  # [128, T, 1]
                xw.reshape(C, T, 128).transpose(2, 1, 0),  # [128, T, C]
            ],
            axis=2,
        ).reshape(128, T * (C + 1))
        oh = np.ascontiguousarray(oh)
        # prior_sigma shard -> pst[p, t*128+z] = psig[z, lo + t*128 + p]
        p = np.zeros((Z, VP), np.float32)
        p[:, :VS] = psig[:, lo:hi]
        pst = np.ascontiguousarray(
            p.reshape(Z, T, 128).transpose(2, 1, 0).reshape(128, T * Z)
        )
        # W_gen shard -> wgt[z, t*128+p] = wgen[lo + t*128 + p, z]
        w = np.zeros((VP, Z), np.float32)
        w[:VS, :] = wgen[lo:hi, :]
        wgt = np.ascontiguousarray(
            w.reshape(T, 128, Z).transpose(2, 0, 1).reshape(Z, T * 128)
        )
        # b_gen shard -> bgt[p, t]; padding gets a huge negative bias so the
        # pad logits can never win the max and exp() maps them to zero.
        b = np.full((VP,), NEG, np.float32)
        b[:VS] = bgen[lo:hi]
        bgt = np.ascontiguousarray(b.reshape(T, 128).T)

        maps.append(
            {
                "et": et,
                "oh": oh,
                "pst": pst,
                "wgt": wgt,
                "bgt": bgt,
                "wmt": wmt,
                "wst": wst,
                "bmu": bmu,
                "bsg": bsg,
                "eps": eps,
                "wgc": wgc,
                "bgc": bgc,
                "idt": idt,
            }
        )
    return maps


def _build():
    nc = bacc.Bacc("TRN2", target_bir_lowering=False, debug=False, num_devices=M)

    et_d = nc.dram_tensor("et", [128, T * D], F32, kind="ExternalInput")
    oh_d = nc.dram_tensor("oh", [128, T * (C + 1)], F32, kind="ExternalInput")
    pst_d = nc.dram_tensor("pst", [128, T * Z], F32, kind="ExternalInput")
    wgt_d = nc.dram_tensor("wgt", [128, T * 128], F32, kind="ExternalInput")
    bgt_d = nc.dram_tensor("bgt", [128, T], F32, kind="ExternalInput")
    wmt_d = nc.dram_tensor("wmt", [128, 768], F32, kind="ExternalInput")
    wst_d = nc.dram_tensor("wst", [128, 768], F32, kind="ExternalInput")
    bmu_d = nc.dram_tensor("bmu", [Z], F32, kind="ExternalInput")
    bsg_d = nc.dram_tensor("bsg", [Z], F32, kind="ExternalInput")
    eps_d = nc.dram_tensor("eps", [Z], F32, kind="ExternalInput")
    wgc_d = nc.dram_tensor("wgc", [Z, C], F32, kind="ExternalInput")
    bgc_d = nc.dram_tensor("bgc", [C], F32, kind="ExternalInput")
    idt_d = nc.dram_tensor("idt", [128, 128], F32, kind="ExternalInput")
    out_d = nc.dram_tensor("out", [1], F32, kind="ExternalOutput")

    ecols = T // ETG * D  # 2100
    pcols = T // PWG * Z  # 896
    rg = [list(range(M))]

    with tile.TileContext(nc) as tc:
        with (
            tc.tile_pool(name="sb", bufs=1) as sb,
            tc.tile_pool(name="ps", bufs=1, space="PSUM") as ps,
            tc.tile_pool(name="dram", bufs=1, space="DRAM") as dram,
        ):
            if WARMUP_CC:
                # Bootstrap the collective stack while input DMAs stream.
                wu_in = dram.tile([8], F32)
                wu_out = dram.tile([M, 8], F32, addr_space="Shared")
                nc.gpsimd.collective_compute(
                    "AllGather",
                    ALU.bypass,
                    ins=[wu_in.opt()],
                    outs=[wu_out.opt()],
                    replica_groups=rg,
                )

            # ---- input DMAs (program order ~ priority) ----
            oh_sb = sb.tile([128, T * (C + 1)], F32)
            nc.sync.dma_start(oh_sb[:], oh_d[:])
            et_sb = []
            for g in range(ETG):
                t_ = sb.tile([128, ecols], F32, name=f"et{g}", tag=f"et{g}")
                nc.sync.dma_start(t_[:], et_d[:, g * ecols : (g + 1) * ecols])
                et_sb.append(t_)
            pst_sb = []
            for g in range(PWG):
                t_ = sb.tile([128, pcols], F32, name=f"pst{g}", tag=f"pst{g}")
                nc.sync.dma_start(t_[:], pst_d[:, g * pcols : (g + 1) * pcols])
                pst_sb.append(t_)
            wmt_sb = sb.tile([128, 768], F32)
            nc.sync.dma_start(wmt_sb[:], wmt_d[:])
            wst_sb = sb.tile([128, 768], F32)
            nc.sync.dma_start(wst_sb[:], wst_d[:])
            bmu_sb = sb.tile([Z, 1], F32)
            nc.sync.dma_start(bmu_sb[:], bmu_d[:])
            bsg_sb = sb.tile([Z, 1], F32)
            nc.sync.dma_start(bsg_sb[:], bsg_d[:])
            eps_sb = sb.tile([Z, 1], F32)
            nc.sync.dma_start(eps_sb[:], eps_d[:])
            wgc_sb = sb.tile([Z, C], F32)
            nc.sync.dma_start(wgc_sb[:], wgc_d[:])
            bgc_sb = sb.tile([C, 1], F32)
            nc.sync.dma_start(bgc_sb[:], bgc_d[:])
            idt_sb = sb.tile([128, 128], F32)
            nc.sync.dma_start(idt_sb[:], idt_d[:])
            bgt_sb = sb.tile([128, T], F32)
            nc.sync.dma_start(bgt_sb[:], bgt_d[:])
            wgt_sb = []
            for g in range(PWG):
                t_ = sb.tile([128, pcols], F32, name=f"wgt{g}", tag=f"wgt{g}")
                nc.sync.dma_start(t_[:], wgt_d[:, g * pcols : (g + 1) * pcols])
                wgt_sb.append(t_)

            ones_sb = sb.tile([128, 1], F32)
            nc.vector.memset(ones_sb[:], 1.0)
            nonesr_sb = sb.tile([1, 128], F32)
            nc.vector.memset(nonesr_sb[:], -1.0)

            # ---- phase A: partial E / prior_sigma contractions ----
            # pA columns: [0:11] d-chunk0, [11:22] d-chunk1, [22:33] d-chunk2
            # (rows 0:44 valid), [33:34] zsig partial.
            pA = ps.tile([128, PACKW], F32)
            for t in range(T):
                g, r = divmod(t, T // ETG)
                rhs = oh_sb[:, t * (C + 1) : (t + 1) * (C + 1)]
                for j, (dlo, dw) in enumerate(DCH):
                    nc.tensor.matmul(
                        pA[0:dw, j * 11 : (j + 1) * 11],
                        et_sb[g][:, r * D + dlo : r * D + dlo + dw],
                        rhs,
                        start=(t == 0),
                        stop=(t == T - 1),
                        skip_group_check=True,
                    )
            for t in range(T):
                g, r = divmod(t, T // PWG)
                nc.tensor.matmul(
                    pA[:, 33:34],
                    pst_sb[g][:, r * Z : (r + 1) * Z],
                    oh_sb[:, t * (C + 1) : t * (C + 1) + 1],
                    start=(t == 0),
                    stop=(t == T - 1),
                    skip_group_check=True,
                )

            pack = sb.tile([128, PACKW], F32)
            nc.vector.memset(pack[:], 0.0)
            nc.vector.tensor_copy(pack[:, 0:22], pA[:, 0:22])
            nc.vector.tensor_copy(pack[0:44, 22:33], pA[0:44, 22:33])
            nc.vector.tensor_copy(pack[:, 33:34], pA[:, 33:34])

            ag1_in = dram.tile([128, PACKW], F32)
            ag1_out = dram.tile([M, 128, PACKW], F32, addr_space="Shared")
            nc.sync.dma_start(ag1_in[:], pack[:])
            nc.gpsimd.collective_compute(
                "AllGather",
                ALU.bypass,
                ins=[ag1_in.opt()],
                outs=[ag1_out.opt()],
                replica_groups=rg,
            )
            agg_sb = sb.tile([128, M, PACKW], F32)
            nc.sync.dma_start(agg_sb[:], ag1_out[:].rearrange("r p f -> p r f"))
            S = sb.tile([128, PACKW], F32)
            nc.vector.reduce_sum(
                S[:], agg_sb[:].rearrange("p r f -> p f r"), axis=mybir.AxisListType.X
            )

            # ---- replicated MLP ----
            R = sb.tile([128, 33], F32)
            nc.scalar.activation(R[:], S[:, 0:33], AF.Relu)
            s_all = sb.tile([128, 6], F32)
            for j in range(3):
                nc.vector.tensor_scalar_mul(
                    s_all[:, j : j + 1], R[:, j * 11 : j * 11 + 1], float(C)
                )
                nc.vector.reduce_sum(
                    s_all[:, 3 + j : 4 + j],
                    R[:, j * 11 + 1 : (j + 1) * 11],
                    axis=mybir.AxisListType.X,
                )
            p_u = ps.tile([Z, 1], F32)
            p_s = ps.tile([Z, 1], F32)
            for j in range(6):
                nc.tensor.matmul(
                    p_u[:],
                    wmt_sb[:, j * 128 : (j + 1) * 128],
                    s_all[:, j : j + 1],
                    start=(j == 0),
                    stop=(j == 5),
                    skip_group_check=True,
                )
                nc.tensor.matmul(
                    p_s[:],
                    wst_sb[:, j * 128 : (j + 1) * 128],
                    s_all[:, j : j + 1],
                    start=(j == 0),
                    stop=(j == 5),
                    skip_group_check=True,
                )
            u_sb = sb.tile([Z, 1], F32)
            nc.vector.tensor_tensor(u_sb[:], p_u[:], bmu_sb[:], op=ALU.add)

            # softplus on both vectors at once: col0 = W_sig pre-act,
            # col1 = prior_sigma lookup.  softplus(x) = relu(x) +
            # ln(1 + exp(-|x|)), with -|x| = min(x, -x) done on DVE.
            spin = sb.tile([Z, 2], F32)
            nc.vector.tensor_tensor(spin[:, 0:1], p_s[:], bsg_sb[:], op=ALU.add)
            nc.vector.tensor_copy(spin[:, 1:2], S[:, 33:34])
            sp_r = sb.tile([Z, 2], F32)
            nc.scalar.activation(sp_r[:], spin[:], AF.Relu)
            sp_n = sb.tile([Z, 2], F32)
            nc.vector.tensor_scalar_mul(sp_n[:], spin[:], -1.0)
            nc.vector.tensor_tensor(sp_n[:], sp_n[:], spin[:], op=ALU.min)
            sp_e = sb.tile([Z, 2], F32)
            nc.scalar.activation(sp_e[:], sp_n[:], AF.Exp)
            nc.vector.tensor_scalar_add(sp_e[:], sp_e[:], 1.0)
            sp_l = sb.tile([Z, 2], F32)
            nc.scalar.activation(sp_l[:], sp_e[:], AF.Ln)
            sp = sb.tile([Z, 2], F32)  # col0 = s, col1 = z_sigma
            nc.vector.tensor_tensor(sp[:], sp_r[:], sp_l[:], op=ALU.add)

            z_sb = sb.tile([Z, 1], F32)
            nc.vector.tensor_tensor(z_sb[:], eps_sb[:], sp[:, 0:1], op=ALU.mult)
            nc.vector.tensor_tensor(z_sb[:], z_sb[:], u_sb[:], op=ALU.add)

            # KL: ln(zs) - ln(s) + (s^2 + (u-zs)^2)/(2 zs^2) - 0.5
            lns = sb.tile([Z, 2], F32)
            nc.scalar.activation(lns[:], sp[:], AF.Ln)
            kl = sb.tile([Z, 1], F32)
            nc.vector.tensor_tensor(
                kl[:], lns[:, 1:2], lns[:, 0:1], op=ALU.subtract
            )
            t1 = sb.tile([Z, 1], F32)
            t2 = sb.tile([Z, 1], F32)
            nc.vector.tensor_tensor(t1[:], u_sb[:], sp[:, 1:2], op=ALU.subtract)
            nc.vector.tensor_tensor(t1[:], t1[:], t1[:], op=ALU.mult)
            nc.vector.tensor_tensor(t2[:], sp[:, 0:1], sp[:, 0:1], op=ALU.mult)
            nc.vector.tensor_tensor(t1[:], t1[:], t2[:], op=ALU.add)
            nc.vector.reciprocal(t2[:], sp[:, 1:2])
            nc.vector.tensor_tensor(t2[:], t2[:], t2[:], op=ALU.mult)
            nc.vector.tensor_tensor(t1[:], t1[:], t2[:], op=ALU.mult)
            nc.vector.tensor_scalar(
                t1[:], t1[:], 0.5, -0.5, op0=ALU.mult, op1=ALU.add
            )
            nc.vector.tensor_tensor(kl[:], kl[:], t1[:], op=ALU.add)
            p_kl = ps.tile([1, 1], F32, tag="tiny")
            nc.tensor.matmul(p_kl[:], kl[:], ones_sb[:], start=True, stop=True)
            klsum = sb.tile([1, 1], F32)
            nc.vector.tensor_copy(klsum[:], p_kl[:])

            # ---- phase B: logits shard ----
            p_l = ps.tile([128, T], F32)
            for t in range(T):
                g, r = divmod(t, T // PWG)
                nc.tensor.matmul(
                    p_l[:, t : t + 1],
                    wgt_sb[g][:, r * 128 : (r + 1) * 128],
                    z_sb[:],
                    start=True,
                    stop=True,
                    skip_group_check=True,
                )
            logits = sb.tile([128, T], F32)
            nc.vector.tensor_tensor(logits[:], p_l[:], bgt_sb[:], op=ALU.add)
            lmaxp = sb.tile([128, 1], F32)
            nc.vector.reduce_max(lmaxp[:], logits[:], axis=mybir.AxisListType.X)
            p_t = ps.tile([1, 128], F32, tag="tiny2")
            nc.tensor.transpose(p_t[:], lmaxp[:], idt_sb[:])
            lmaxr = sb.tile([1, 128], F32)
            nc.vector.tensor_copy(lmaxr[:], p_t[:])
            lmax = sb.tile([1, 1], F32)
            nc.vector.reduce_max(lmax[:], lmaxr[:], axis=mybir.AxisListType.X)
            # broadcast -lmax to all partitions via PE (lhsT = -ones row)
            p_b = ps.tile([128, 1], F32, tag="tiny")
            nc.tensor.matmul(p_b[:], nonesr_sb[:], lmax[:], start=True, stop=True)
            nlb = sb.tile([128, 1], F32)
            nc.vector.tensor_copy(nlb[:], p_b[:])
            ex = sb.tile([128, T], F32)
            esum = sb.tile([128, 1], F32)
            nc.scalar.activation(
                ex[:], logits[:], AF.Exp, bias=nlb[:], accum_out=esum[:]
            )
            p_e = ps.tile([1, 1], F32, tag="tiny2")
            nc.tensor.matmul(p_e[:], esum[:], ones_sb[:], start=True, stop=True)

            pair = sb.tile([1, 8], F32)
            nc.vector.memset(pair[:], 0.0)
            nc.vector.tensor_copy(pair[:, 0:1], lmax[:])
            nc.vector.tensor_copy(pair[:, 1:2], p_e[:])
            ag2_in = dram.tile([8], F32)
            ag2_out = dram.tile([M, 8], F32, addr_space="Shared")
            nc.sync.dma_start(ag2_in[:], pair[:])
            nc.gpsimd.collective_compute(
                "AllGather",
                ALU.bypass,
                ins=[ag2_in.opt()],
                outs=[ag2_out.opt()],
                replica_groups=rg,
            )
            agp = sb.tile([1, M * 8], F32)
            nc.sync.dma_start(agp[:], ag2_out[:].rearrange("r f -> (r f)"))

            gmax = sb.tile([1, 1], F32)
            nc.vector.reduce_max(
                gmax[:], agp[:, 0 : M * 8 : 8], axis=mybir.AxisListType.X
            )
            w8 = sb.tile([1, M], F32)
            nc.vector.tensor_scalar(
                w8[:], agp[:, 0 : M * 8 : 8], gmax[:], None, op0=ALU.subtract
            )
            nc.scalar.activation(w8[:], w8[:], AF.Exp)
            nc.vector.tensor_tensor(
                w8[:], w8[:], agp[:, 1 : M * 8 : 8], op=ALU.mult
            )
            gsum = sb.tile([1, 1], F32)
            nc.vector.reduce_sum(gsum[:], w8[:], axis=mybir.AxisListType.X)

            # ---- context logits from host-gathered W_gen rows ----
            p_c = ps.tile([C, 1], F32, tag="tiny")
            nc.tensor.matmul(p_c[:], wgc_sb[:], z_sb[:], start=True, stop=True)
            cl = sb.tile([C, 1], F32)
            nc.vector.tensor_tensor(cl[:], p_c[:], bgc_sb[:], op=ALU.add)
            p_cs = ps.tile([1, 1], F32, tag="tiny2")
            nc.tensor.matmul(p_cs[:], cl[:], ones_sb[0:C, :], start=True, stop=True)

            # ---- final scalar ----
            res = sb.tile([1, 1], F32)
            nc.scalar.activation(res[:], gsum[:], AF.Ln)
            nc.vector.tensor_tensor(res[:], res[:], gmax[:], op=ALU.add)
            nc.vector.tensor_scalar_mul(res[:], res[:], float(C))
            tfin = sb.tile([1, 1], F32)
            nc.vector.tensor_tensor(tfin[:], p_cs[:], res[:], op=ALU.subtract)
            nc.vector.tensor_tensor(tfin[:], tfin[:], klsum[:], op=ALU.subtract)
            nc.sync.dma_start(out_d[:], tfin[:])

    nc.compile()
    return nc


_NC = None
_EXEC = None


def _get_exec():
    """Build the jit'd 8-device SPMD callable once."""
    global _NC, _EXEC
    if _EXEC is not None:
        return _EXEC
    import jax
    from jax.experimental.shard_map import shard_map
    from jax.sharding import Mesh, NamedSharding, PartitionSpec

    from concourse import bass2jax

    if _NC is None:
        _NC = _build()
    nc = _NC
    bass2jax.install_neuronx_cc_hook()

    partition_name = nc.partition_id_tensor.name if nc.partition_id_tensor else None
    in_names, out_names, out_avals = [], [], []
    for alloc in nc.m.functions[0].allocations:
        if not isinstance(alloc, mybir.MemoryLocationSet):
            continue
        name = alloc.memorylocations[0].name
        if alloc.kind == "ExternalInput":
            if name != partition_name:
                in_names.append(name)
        elif alloc.kind == "ExternalOutput":
            shape = tuple(alloc.tensor_shape)
            dtype = mybir.dt.np(alloc.dtype)
            out_names.append(name)
            out_avals.append(jax.core.ShapedArray(shape, dtype))
    n_params = len(in_names)
    n_outs = len(out_names)
    all_in_names = list(in_names) + list(out_names)
    if partition_name is not None:
        all_in_names.append(partition_name)

    def _body(*args):
        operands = list(args)
        if partition_name is not None:
            operands.append(bass2jax.partition_id_tensor())
        outs = bass2jax._bass_exec_p.bind(
            *operands,
            out_avals=tuple(out_avals),
            in_names=tuple(all_in_names),
            out_names=tuple(out_names),
            lowering_input_output_aliases=(),
            sim_require_finite=True,
            sim_require_nnan=True,
            nc=nc,
        )
        return tuple(outs)

    devices = jax.devices()[:M]
    mesh = Mesh(np.asarray(devices), ("core",))
    donate = tuple(range(n_params, n_params + n_outs))
    sharded = jax.jit(
        shard_map(
            _body,
            mesh=mesh,
            in_specs=(PartitionSpec("core"),) * (n_params + n_outs),
            out_specs=(PartitionSpec("core"),) * n_outs,
            check_rep=False,
        ),
        donate_argnums=donate,
        keep_unused=True,
    )
    sh = NamedSharding(mesh, PartitionSpec("core"))
    _EXEC = (sharded, in_names, out_names, out_avals, sh)
    return _EXEC


def _run(in_maps, trace=False):
    """Execute with inputs pre-staged on the devices so all 8 ranks start
    aligned.  Returns (per-core results, exec_time_ns, profile_json)."""
    import jax

    sharded, in_names, out_names, out_avals, sh = _get_exec()
    nc = _NC
    concat_in = [
        np.concatenate([np.asarray(m[n]) for m in in_maps], axis=0)
        for n in in_names
    ]
    staged = [jax.device_put(a, sh) for a in concat_in]
    zeros = [
        jax.device_put(
            np.zeros((M * av.shape[0], *av.shape[1:]), av.dtype), sh
        )
        for av in out_avals
    ]
    jax.block_until_ready(staged)
    jax.block_until_ready(zeros)

    exec_time_ns = None
    profile_json = None
    if trace:
        try:
            from antenv.axon_hooks import get_axon_ntff_profile_hook

            hook = get_axon_ntff_profile_hook()
        except Exception:
            hook = None
        if hook is not None:
            import gauge.profiler

            bass_utils.upload_artifacts = lambda tmpdir: "local://skipped"
            td = tempfile.mkdtemp()
            with hook(td, [0]):
                out_arrs = sharded(*staged, *zeros)
                jax.block_until_ready(out_arrs)
            ntffs = glob.glob(os.path.join(td, "*_body*.ntff"))
            if ntffs:
                profile = gauge.profiler.Profile(
                    profile_path=bass_utils.FishPath(td),
                    kernel_dev_mode=True,
                    profile_on_exit=False,
                    bass_kernel=nc.m,
                    offline_processing=True,
                    fname="*_body*",
                    metadata={"artifacts_path": "local://skipped"},
                )
                perf = bass_utils._process_ntff_profile(
                    profile, td, nc, list(range(M)), None, False, {}, False
                )
                exec_time_ns = perf.exec_time_ns
                profile_json = perf.profile_json
        else:
            out_arrs = sharded(*staged, *zeros)
            jax.block_until_ready(out_arrs)
    else:
        out_arrs = sharded(*staged, *zeros)
        jax.block_until_ready(out_arrs)

    results = [
        {
            name: np.asarray(out_arrs[i]).reshape(M, *out_avals[i].shape)[c]
            for i, name in enumerate(out_names)
        }
        for c in range(M)
    ]
    return results, exec_time_ns, profile_json


def kernel(**inputs) -> np.ndarray:
    in_maps = _shard_inputs(inputs)
    trace = bool(os.environ.get("KERNEL_TRACE"))
    repeat = int(os.environ.get("KERNEL_REPEAT", "1"))
    for _ in range(repeat - 1):
        _run(in_maps, trace=False)
    results, exec_ns, prof = _run(in_maps, trace=trace)
    kernel.last_exec_time_ns = exec_ns
    kernel.last_profile_json = prof
    return np.asarray(np.float32(results[0]["out"][0])).reshape(())


def emulate(**inputs) -> np.ndarray:
    """Numpy emulation of the exact device dataflow (for layout validation)."""
    maps = _shard_inputs(inputs)
    packs = []
    for m in range(M):
        mp = maps[m]
        et, oh, pst = mp["et"], mp["oh"], mp["pst"]
        pack = np.zeros((128, PACKW), np.float32)
        for t in range(T):
            rhs = oh[:, t * (C + 1) : (t + 1) * (C + 1)]  # [128, 11]
            for j, (dlo, dw) in enumerate(DCH):
                lhsT = et[:, t * D + dlo : t * D + dlo + dw]  # [128, dw]
                pack[0:dw, j * 11 : (j + 1) * 11] += lhsT.T @ rhs
            lhsT = pst[:, t * Z : (t + 1) * Z]
            pack[:, 33] += lhsT.T @ oh[:, t * (C + 1)]
        packs.append(pack)
    S = np.sum(packs, axis=0)  # AllGather + local reduce
    R = np.maximum(S[:, 0:33], 0.0)
    s_all = np.zeros((128, 6), np.float32)
    for j in range(3):
        s_all[:, j] = C * R[:, j * 11]
        s_all[:, 3 + j] = R[:, j * 11 + 1 : (j + 1) * 11].sum(axis=1)
    mp = maps[0]

    def mlp(wmt):
        acc = np.zeros((Z,), np.float32)
        for j in range(6):
            acc += wmt[:, j * 128 : (j + 1) * 128].T @ s_all[:, j]
        return acc

    def sp(x):
        return np.maximum(x, 0) + np.log1p(np.exp(-np.abs(x)))

    u = mlp(mp["wmt"]) + mp["bmu"]
    s = sp(mlp(mp["wst"]) + mp["bsg"])
    z = u + mp["eps"] * s
    zs = sp(S[:, 33])
    kl = np.log(zs) - np.log(s) + (s**2 + (u - zs) ** 2) * 0.5 / zs**2 - 0.5
    klsum = kl.sum()

    pairs = []
    for m in range(M):
        mp_ = maps[m]
        logits = np.zeros((128, T), np.float32)
        for t in range(T):
            logits[:, t] = mp_["wgt"][:, t * 128 : (t + 1) * 128].T @ z
        logits += mp_["bgt"]
        lmax = logits.max()
        esum = np.exp(logits - lmax).sum()
        pairs.append((lmax, esum))
    gmax = max(p[0] for p in pairs)
    gsum = sum(p[1] * np.exp(p[0] - gmax) for p in pairs)
    cl = mp["wgc"].T @ z + mp["bgc"]
    resv = cl.sum() - C * (gmax + np.log(gsum)) - klsum
    return np.asarray(np.float32(resv)).reshape(())


# revision 4
# speedup vs baseline: 1.5513x; 1.0689x over previous
"""Bayesian SkipGram forward pass on 8 Trainium2 cores.

Strategy (vocab/model parallel, per the V-axis sharding):
  - V=50000 is split into 8 shards of 6250, each padded to 6272 = 49*128.
  - Each core holds its shard of [E ; prior_sigma] (transposed and
    interleaved per 128-wide v-tile), W_gen (transposed) and b_gen, plus
    replicated copies of the tiny Z/2D-sized tensors.
  - Phase A (per core): one matmul per v-tile with the 11 one-hot columns
    (center + 10 context words) as the stationary operand and the
    [300 E | 128 prior_sigma] block as the moving operand, accumulating
    partial lookups in PSUM.  One small AllGather combines the 8 partial
    blocks; every core reduces them locally.
  - Replicated MLP: relu/sums -> summed, u/s via 6 matmuls with the summed
    chunks stationary (streaming [W_mu | W_sig]), softplus, z = u + eps*s,
    and the KL terms -- all in [1, 128] row form so reductions stay on the
    vector engine.
  - Phase B: z is the stationary operand (loaded once); W_gen streams
    through 512 columns at a time producing flat logits, which are
    scattered to [128, 49] via a DRAM bounce for lane-parallel max/exp.
    A second tiny AllGather of (local_max, local_sumexp) pairs gives every
    core the exact global log_softmax denominator.
  - loss_probs gather: logits at context_word_idxs are recomputed exactly
    from host-gathered rows W_gen[idxs, :] (index gather, done once on the
    host) so no cross-shard index traffic is needed.
  - prior_mean is unused by the reference model and is never transferred.
  - A dummy AllGather issued at kernel start absorbs the collective
    communicator bootstrap concurrently with the input DMA phase.

The final scalar is computed redundantly on every core; core 0's output is
returned.  Inputs are pre-staged onto the 8 devices (device_put + block)
before the NEFF executes so all ranks start aligned.
"""

import glob
import os
import sys
import tempfile
import types

import numpy as np


def _install_ntff_hook():
    """Fail-soft shim: the agent image's antenv lacks axon_hooks, which
    bass_utils imports when tracing is requested."""
    try:
        if "antenv.axon_hooks" in sys.modules:
            return
        import antenv

        mod = types.ModuleType("antenv.axon_hooks")
        mod._hook = None

        def set_axon_ntff_profile_hook(h):
            mod._hook = h

        def get_axon_ntff_profile_hook():
            return mod._hook

        mod.set_axon_ntff_profile_hook = set_axon_ntff_profile_hook
        mod.get_axon_ntff_profile_hook = get_axon_ntff_profile_hook
        sys.modules["antenv.axon_hooks"] = mod
        antenv.axon_hooks = mod
        try:
            from trn_agent_boot.trn_boot import _ntff_profile_via_ctypes

            set_axon_ntff_profile_hook(
                _ntff_profile_via_ctypes("/opt/axon/libaxon_pjrt.so")
            )
        except Exception:
            pass
    except Exception:
        pass


_install_ntff_hook()

import concourse.bacc as bacc
import concourse.bass_utils as bass_utils
import concourse.mybir as mybir
import concourse.tile as tile

V, D, Z, C = 50000, 300, 128, 10
M = 8  # cores
VS = V // M  # 6250 real elements per shard
T = 49  # 128-wide v-tiles per shard
VP = T * 128  # 6272 padded shard size
EB = D + Z  # 428: columns per v-tile block of [E | prior_sigma]
ETG = 7  # [E|psig] tile split (7 v-tiles each) for DMA/compute overlap
PWG = 7  # W_gen tile split
F32 = mybir.dt.float32
AF = mybir.ActivationFunctionType
ALU = mybir.AluOpType
NEG = -1.0e30
WARMUP_CC = True


def _shard_inputs(inputs):
    """Host-side: slice/pad/transpose the full tensors into per-core device
    layouts.  Returns list of 8 in_maps."""
    E = np.asarray(inputs["E"], np.float32)
    psig = np.asarray(inputs["prior_sigma"], np.float32)
    wgen = np.asarray(inputs["W_gen"], np.float32)
    bgen = np.asarray(inputs["b_gen"], np.float32)
    center = np.asarray(inputs["center_word"], np.float32)
    ctx = np.asarray(inputs["context_words"], np.float32)
    idxs = np.asarray(inputs["context_word_idxs"]).astype(np.int64)

    wmu = np.asarray(inputs["W_mu"], np.float32)
    wsig = np.asarray(inputs["W_sig"], np.float32)

    # wms[p, j*256 + 0:128] = W_mu[z, j*128+p]; [128:256] likewise W_sig,
    # with the 600 summed-dim entries laid out as two zero-padded 384 halves.
    def pad_mlp(w):  # [Z, 600] -> [768, Z]
        out = np.zeros((Z, 768), np.float32)
        out[:, 0:300] = w[:, 0:300]
        out[:, 384:684] = w[:, 300:600]
        return out.T  # [dcol, z]

    wmp = pad_mlp(wmu).reshape(6, 128, Z)
    wsp = pad_mlp(wsig).reshape(6, 128, Z)
    wms = np.ascontiguousarray(
        np.concatenate([wmp, wsp], axis=2).transpose(1, 0, 2).reshape(128, 6 * 256)
    )
    bmu = np.ascontiguousarray(np.asarray(inputs["b_mu"], np.float32))
    bsg = np.ascontiguousarray(np.asarray(inputs["b_sig"], np.float32))
    eps = np.ascontiguousarray(np.asarray(inputs["eps"], np.float32))
    wgc = np.ascontiguousarray(wgen[idxs, :].T)  # [Z, C]
    bgc = np.ascontiguousarray(bgen[idxs])  # [C]
    idt = np.eye(128, dtype=np.float32)

    maps = []
    for m in range(M):
        lo = m * VS
        hi = lo + VS
        # [E | prior_sigma] shard:
        # etp[p, t*EB + d]     = E[d, lo + t*128 + p]        (d < 300)
        # etp[p, t*EB + 300+z] = psig[z, lo + t*128 + p]
        e = np.zeros((D, VP), np.float32)
        e[:, :VS] = E[:, lo:hi]
        p = np.zeros((Z, VP), np.float32)
        p[:, :VS] = psig[:, lo:hi]
        ep = np.concatenate([e, p], axis=0)  # [EB, VP]
        etp = np.ascontiguousarray(
            ep.reshape(EB, T, 128).transpose(2, 1, 0).reshape(128, T * EB)
        )
        # one-hots -> oh[p, t*11+0]=center, [p, t*11+1+c]=ctx[c]
        cw = np.zeros((VP,), np.float32)
        cw[:VS] = center[lo:hi]
        xw = np.zeros((C, VP), np.float32)
        xw[:, :VS] = ctx[:, lo:hi]
        oh = np.concatenate(
            [
                cw.reshape(T, 128).T[:, :, None],  # [128, T, 1]
                xw.reshape(C, T, 128).transpose(2, 1, 0),  # [128, T, C]
            ],
            axis=2,
        ).reshape(128, T * (C + 1))
        oh = np.ascontiguousarray(oh)
        # W_gen shard -> wgt[z, t*128+p] = wgen[lo + t*128 + p, z]
        w = np.zeros((VP, Z), np.float32)
        w[:VS, :] = wgen[lo:hi, :]
        wgt = np.ascontiguousarray(
            w.reshape(T, 128, Z).transpose(2, 0, 1).reshape(Z, T * 128)
        )
        # b_gen shard -> bgt[p, t]; padding gets a huge negative bias so the
        # pad logits can never win the max and exp() maps them to zero.
        b = np.full((VP,), NEG, np.float32)
        b[:VS] = bgen[lo:hi]
        bgt = np.ascontiguousarray(b.reshape(T, 128).T)

        maps.append(
            {
                "etp": etp,
                "oh": oh,
                "wgt": wgt,
                "bgt": bgt,
                "wms": wms,
                "bmu": bmu,
                "bsg": bsg,
                "eps": eps,
                "wgc": wgc,
                "bgc": bgc,
                "idt": idt,
            }
        )
    return maps


def _build():
    nc = bacc.Bacc("TRN2", target_bir_lowering=False, debug=False, num_devices=M)

    etp_d = nc.dram_tensor("etp", [128, T * EB], F32, kind="ExternalInput")
    oh_d = nc.dram_tensor("oh", [128, T * (C + 1)], F32, kind="ExternalInput")
    wgt_d = nc.dram_tensor("wgt", [128, T * 128], F32, kind="ExternalInput")
    bgt_d = nc.dram_tensor("bgt", [128, T], F32, kind="ExternalInput")
    wms_d = nc.dram_tensor("wms", [128, 6 * 256], F32, kind="ExternalInput")
    bmu_d = nc.dram_tensor("bmu", [Z], F32, kind="ExternalInput")
    bsg_d = nc.dram_tensor("bsg", [Z], F32, kind="ExternalInput")
    eps_d = nc.dram_tensor("eps", [Z], F32, kind="ExternalInput")
    wgc_d = nc.dram_tensor("wgc", [Z, C], F32, kind="ExternalInput")
    bgc_d = nc.dram_tensor("bgc", [C], F32, kind="ExternalInput")
    idt_d = nc.dram_tensor("idt", [128, 128], F32, kind="ExternalInput")
    out_d = nc.dram_tensor("out", [1], F32, kind="ExternalOutput")

    ecols = T // ETG * EB  # 2996
    pcols = T // PWG * 128  # 896
    rg = [list(range(M))]

    with tile.TileContext(nc) as tc:
        with (
            tc.tile_pool(name="sb", bufs=1) as sb,
            tc.tile_pool(name="ps", bufs=1, space="PSUM") as ps,
            tc.tile_pool(name="dram", bufs=1, space="DRAM") as dram,
        ):
            if WARMUP_CC:
                # Bootstrap the collective stack while input DMAs stream.
                wu_in = dram.tile([8], F32)
                wu_out = dram.tile([M, 8], F32, addr_space="Shared")
                nc.gpsimd.collective_compute(
                    "AllGather",
                    ALU.bypass,
                    ins=[wu_in.opt()],
                    outs=[wu_out.opt()],
                    replica_groups=rg,
                )

            # ---- input DMAs (program order ~ priority) ----
            oh_sb = sb.tile([128, T * (C + 1)], F32)
            nc.sync.dma_start(oh_sb[:], oh_d[:])
            etp_sb = []
            for g in range(ETG):
                t_ = sb.tile([128, ecols], F32, name=f"etp{g}", tag=f"etp{g}")
                nc.sync.dma_start(t_[:], etp_d[:, g * ecols : (g + 1) * ecols])
                etp_sb.append(t_)
            wms_sb = sb.tile([128, 6 * 256], F32)
            nc.sync.dma_start(wms_sb[:], wms_d[:])
            bmu_sb = sb.tile([1, Z], F32)
            nc.sync.dma_start(bmu_sb[:], bmu_d[:])
            bsg_sb = sb.tile([1, Z], F32)
            nc.sync.dma_start(bsg_sb[:], bsg_d[:])
            eps_sb = sb.tile([1, Z], F32)
            nc.sync.dma_start(eps_sb[:], eps_d[:])
            wgc_sb = sb.tile([Z, C], F32)
            nc.sync.dma_start(wgc_sb[:], wgc_d[:])
            bgc_sb = sb.tile([1, C], F32)
            nc.sync.dma_start(bgc_sb[:], bgc_d[:])
            idt_sb = sb.tile([128, 128], F32)
            nc.sync.dma_start(idt_sb[:], idt_d[:])
            bgt_sb = sb.tile([128, T], F32)
            nc.sync.dma_start(bgt_sb[:], bgt_d[:])
            wgt_sb = []
            for g in range(PWG):
                t_ = sb.tile([128, pcols], F32, name=f"wgt{g}", tag=f"wgt{g}")
                nc.sync.dma_start(t_[:], wgt_d[:, g * pcols : (g + 1) * pcols])
                wgt_sb.append(t_)

            ones_sb = sb.tile([128, 1], F32)
            nc.vector.memset(ones_sb[:], 1.0)
            nonesr_sb = sb.tile([1, 128], F32)
            nc.vector.memset(nonesr_sb[:], -1.0)

            # ---- phase A: one matmul per v-tile ----
            # pA[c, 0:300]   = partial E lookups (c=0 center, 1..10 ctx)
            # pA[c, 300:428] = partial prior_sigma lookups (row 0 is used)
            pA = ps.tile([C + 1, EB], F32)
            for t in range(T):
                g, r = divmod(t, T // ETG)
                nc.tensor.matmul(
                    pA[:],
                    oh_sb[:, t * (C + 1) : (t + 1) * (C + 1)],
                    etp_sb[g][:, r * EB : (r + 1) * EB],
                    start=(t == 0),
                    stop=(t == T - 1),
                )

            pack = sb.tile([C + 1, EB], F32)
            nc.vector.tensor_copy(pack[:], pA[:])
            ag1_in = dram.tile([C + 1, EB], F32)
            ag1_out = dram.tile([M, C + 1, EB], F32, addr_space="Shared")
            nc.sync.dma_start(ag1_in[:], pack[:])
            nc.gpsimd.collective_compute(
                "AllGather",
                ALU.bypass,
                ins=[ag1_in.opt()],
                outs=[ag1_out.opt()],
                replica_groups=rg,
            )
            agg_sb = sb.tile([C + 1, M, EB], F32)
            nc.sync.dma_start(agg_sb[:], ag1_out[:].rearrange("r p f -> p r f"))
            S = sb.tile([C + 1, EB], F32)
            nc.vector.reduce_sum(
                S[:], agg_sb[:].rearrange("p r f -> p f r"), axis=mybir.AxisListType.X
            )

            # ---- replicated MLP (row form) ----
            R = sb.tile([C + 1, D], F32)
            nc.scalar.activation(R[:], S[:, 0:D], AF.Relu)
            # sum over the 11 rows via PE; context-sum = all-rows - center row
            p_s2 = ps.tile([1, D], F32, tag="tiny")
            nc.tensor.matmul(
                p_s2[:], ones_sb[0 : C + 1, :], R[:], start=True, stop=True
            )
            srow = sb.tile([1, 768], F32)
            nc.vector.memset(srow[:], 0.0)
            nc.vector.tensor_scalar_mul(srow[:, 0:D], R[0:1, :], float(C))
            nc.vector.tensor_tensor(
                srow[:, 384 : 384 + D], p_s2[:], R[0:1, :], op=ALU.subtract
            )
            # scatter summed row into column form via DRAM bounce
            sd = dram.tile([768], F32)
            nc.sync.dma_start(sd[:], srow[:])
            scol = sb.tile([128, 6], F32)
            nc.sync.dma_start(scol[:], sd[:].rearrange("(j p) -> p j", p=128))
            # u/s: 6 matmuls, summed chunks stationary, [W_mu | W_sig] moving
            p_us = ps.tile([1, 256], F32, tag="tiny2")
            for j in range(6):
                nc.tensor.matmul(
                    p_us[:],
                    scol[:, j : j + 1],
                    wms_sb[:, j * 256 : (j + 1) * 256],
                    start=(j == 0),
                    stop=(j == 5),
                )
            u_sb = sb.tile([1, Z], F32)
            nc.vector.tensor_tensor(u_sb[:], p_us[:, 0:Z], bmu_sb[:], op=ALU.add)

            # softplus on both vectors at once: [0:128] = W_sig pre-act,
            # [128:256] = prior_sigma lookup.  softplus(x) = relu(x) +
            # ln(1 + exp(-|x|)), with -|x| = min(x, -x) done on DVE.
            spin = sb.tile([1, 2 * Z], F32)
            nc.vector.tensor_tensor(
                spin[:, 0:Z], p_us[:, Z : 2 * Z], bsg_sb[:], op=ALU.add
            )
            nc.vector.tensor_copy(spin[:, Z : 2 * Z], S[0:1, D:EB])
            sp_r = sb.tile([1, 2 * Z], F32)
            nc.scalar.activation(sp_r[:], spin[:], AF.Relu)
            sp_n = sb.tile([1, 2 * Z], F32)
            nc.vector.tensor_scalar_mul(sp_n[:], spin[:], -1.0)
            nc.vector.tensor_tensor(sp_n[:], sp_n[:], spin[:], op=ALU.min)
            sp_e = sb.tile([1, 2 * Z], F32)
            nc.scalar.activation(sp_e[:], sp_n[:], AF.Exp)
            nc.vector.tensor_scalar_add(sp_e[:], sp_e[:], 1.0)
            sp_l = sb.tile([1, 2 * Z], F32)
            nc.scalar.activation(sp_l[:], sp_e[:], AF.Ln)
            sp = sb.tile([1, 2 * Z], F32)  # [0:128] = s, [128:256] = z_sigma
            nc.vector.tensor_tensor(sp[:], sp_r[:], sp_l[:], op=ALU.add)

            z_row = sb.tile([1, Z], F32)
            nc.vector.tensor_tensor(z_row[:], eps_sb[:], sp[:, 0:Z], op=ALU.mult)
            nc.vector.tensor_tensor(z_row[:], z_row[:], u_sb[:], op=ALU.add)
            zd = dram.tile([Z], F32)
            nc.sync.dma_start(zd[:], z_row[:])
            zcol = sb.tile([Z, 1], F32)
            nc.sync.dma_start(zcol[:], zd[:])

            # KL: ln(zs) - ln(s) + (s^2 + (u-zs)^2)/(2 zs^2) - 0.5, then sum
            lns = sb.tile([1, 2 * Z], F32)
            nc.scalar.activation(lns[:], sp[:], AF.Ln)
            kl = sb.tile([1, Z], F32)
            nc.vector.tensor_tensor(
                kl[:], lns[:, Z : 2 * Z], lns[:, 0:Z], op=ALU.subtract
            )
            t1 = sb.tile([1, Z], F32)
            t2 = sb.tile([1, Z], F32)
            nc.vector.tensor_tensor(t1[:], u_sb[:], sp[:, Z : 2 * Z], op=ALU.subtract)
            nc.vector.tensor_tensor(t1[:], t1[:], t1[:], op=ALU.mult)
            nc.vector.tensor_tensor(t2[:], sp[:, 0:Z], sp[:, 0:Z], op=ALU.mult)
            nc.vector.tensor_tensor(t1[:], t1[:], t2[:], op=ALU.add)
            nc.vector.reciprocal(t2[:], sp[:, Z : 2 * Z])
            nc.vector.tensor_tensor(t2[:], t2[:], t2[:], op=ALU.mult)
            nc.vector.tensor_tensor(t1[:], t1[:], t2[:], op=ALU.mult)
            nc.vector.tensor_scalar(t1[:], t1[:], 0.5, -0.5, op0=ALU.mult, op1=ALU.add)
            nc.vector.tensor_tensor(kl[:], kl[:], t1[:], op=ALU.add)
            klsum = sb.tile([1, 1], F32)
            nc.vector.reduce_sum(klsum[:], kl[:], axis=mybir.AxisListType.X)

            # ---- phase B: logits shard, z stationary ----
            lflat = sb.tile([1, VP], F32)
            for g in range(PWG):
                for h, (clo, cw_) in enumerate([(0, 512), (512, 384)]):
                    p_l = ps.tile(
                        [1, 512], F32, name=f"pl{g}_{h}", tag="plx", bufs=2
                    )
                    nc.tensor.matmul(
                        p_l[:, 0:cw_],
                        zcol[:],
                        wgt_sb[g][:, clo : clo + cw_],
                        start=True,
                        stop=True,
                    )
                    nc.vector.tensor_copy(
                        lflat[:, g * pcols + clo : g * pcols + clo + cw_],
                        p_l[:, 0:cw_],
                    )
            ld = dram.tile([VP], F32)
            nc.sync.dma_start(ld[:], lflat[:])
            lg = sb.tile([128, T], F32)
            nc.sync.dma_start(lg[:], ld[:].rearrange("(t p) -> p t", p=128))
            logits = sb.tile([128, T], F32)
            nc.vector.tensor_tensor(logits[:], lg[:], bgt_sb[:], op=ALU.add)
            lmaxp = sb.tile([128, 1], F32)
            nc.vector.reduce_max(lmaxp[:], logits[:], axis=mybir.AxisListType.X)
            p_t = ps.tile([1, 128], F32, tag="tiny")
            nc.tensor.transpose(p_t[:], lmaxp[:], idt_sb[:])
            lmaxr = sb.tile([1, 128], F32)
            nc.vector.tensor_copy(lmaxr[:], p_t[:])
            lmax = sb.tile([1, 1], F32)
            nc.vector.reduce_max(lmax[:], lmaxr[:], axis=mybir.AxisListType.X)
            # broadcast -lmax to all partitions via PE (lhsT = -ones row)
            p_b = ps.tile([128, 1], F32, tag="tiny2")
            nc.tensor.matmul(p_b[:], nonesr_sb[:], lmax[:], start=True, stop=True)
            nlb = sb.tile([128, 1], F32)
            nc.vector.tensor_copy(nlb[:], p_b[:])
            ex = sb.tile([128, T], F32)
            esum = sb.tile([128, 1], F32)
            nc.scalar.activation(
                ex[:], logits[:], AF.Exp, bias=nlb[:], accum_out=esum[:]
            )
            p_e = ps.tile([1, 1], F32, tag="tiny")
            nc.tensor.matmul(p_e[:], esum[:], ones_sb[:], start=True, stop=True)

            pair = sb.tile([1, 8], F32)
            nc.vector.memset(pair[:], 0.0)
            nc.vector.tensor_copy(pair[:, 0:1], lmax[:])
            nc.vector.tensor_copy(pair[:, 1:2], p_e[:])
            ag2_in = dram.tile([8], F32)
            ag2_out = dram.tile([M, 8], F32, addr_space="Shared")
            nc.sync.dma_start(ag2_in[:], pair[:])
            nc.gpsimd.collective_compute(
                "AllGather",
                ALU.bypass,
                ins=[ag2_in.opt()],
                outs=[ag2_out.opt()],
                replica_groups=rg,
            )
            agp = sb.tile([1, M * 8], F32)
            nc.sync.dma_start(agp[:], ag2_out[:].rearrange("r f -> (r f)"))

            gmax = sb.tile([1, 1], F32)
            nc.vector.reduce_max(
                gmax[:], agp[:, 0 : M * 8 : 8], axis=mybir.AxisListType.X
            )
            w8 = sb.tile([1, M], F32)
            nc.vector.tensor_scalar(
                w8[:], agp[:, 0 : M * 8 : 8], gmax[:], None, op0=ALU.subtract
            )
            nc.scalar.activation(w8[:], w8[:], AF.Exp)
            nc.vector.tensor_tensor(
                w8[:], w8[:], agp[:, 1 : M * 8 : 8], op=ALU.mult
            )
            gsum = sb.tile([1, 1], F32)
            nc.vector.reduce_sum(gsum[:], w8[:], axis=mybir.AxisListType.X)

            # ---- context logits from host-gathered W_gen rows ----
            p_c = ps.tile([1, C], F32, tag="tiny2")
            nc.tensor.matmul(p_c[:], zcol[:], wgc_sb[:], start=True, stop=True)
            cl = sb.tile([1, C], F32)
            nc.vector.tensor_tensor(cl[:], p_c[:], bgc_sb[:], op=ALU.add)
            csum = sb.tile([1, 1], F32)
            nc.vector.reduce_sum(csum[:], cl[:], axis=mybir.AxisListType.X)

            # ---- final scalar ----
            res = sb.tile([1, 1], F32)
            nc.scalar.activation(res[:], gsum[:], AF.Ln)
            nc.vector.tensor_tensor(res[:], res[:], gmax[:], op=ALU.add)
            nc.vector.tensor_scalar_mul(res[:], res[:], float(C))
            tfin = sb.tile([1, 1], F32)
            nc.vector.tensor_tensor(tfin[:], csum[:], res[:], op=ALU.subtract)
            nc.vector.tensor_tensor(tfin[:], tfin[:], klsum[:], op=ALU.subtract)
            nc.sync.dma_start(out_d[:], tfin[:])

    nc.compile()
    return nc


_NC = None
_EXEC = None


def _get_exec():
    """Build the jit'd 8-device SPMD callable once."""
    global _NC, _EXEC
    if _EXEC is not None:
        return _EXEC
    import jax
    from jax.experimental.shard_map import shard_map
    from jax.sharding import Mesh, NamedSharding, PartitionSpec

    from concourse import bass2jax

    if _NC is None:
        _NC = _build()
    nc = _NC
    bass2jax.install_neuronx_cc_hook()

    partition_name = nc.partition_id_tensor.name if nc.partition_id_tensor else None
    in_names, out_names, out_avals = [], [], []
    for alloc in nc.m.functions[0].allocations:
        if not isinstance(alloc, mybir.MemoryLocationSet):
            continue
        name = alloc.memorylocations[0].name
        if alloc.kind == "ExternalInput":
            if name != partition_name:
                in_names.append(name)
        elif alloc.kind == "ExternalOutput":
            shape = tuple(alloc.tensor_shape)
            dtype = mybir.dt.np(alloc.dtype)
            out_names.append(name)
            out_avals.append(jax.core.ShapedArray(shape, dtype))
    n_params = len(in_names)
    n_outs = len(out_names)
    all_in_names = list(in_names) + list(out_names)
    if partition_name is not None:
        all_in_names.append(partition_name)

    def _body(*args):
        operands = list(args)
        if partition_name is not None:
            operands.append(bass2jax.partition_id_tensor())
        outs = bass2jax._bass_exec_p.bind(
            *operands,
            out_avals=tuple(out_avals),
            in_names=tuple(all_in_names),
            out_names=tuple(out_names),
            lowering_input_output_aliases=(),
            sim_require_finite=True,
            sim_require_nnan=True,
            nc=nc,
        )
        return tuple(outs)

    devices = jax.devices()[:M]
    mesh = Mesh(np.asarray(devices), ("core",))
    donate = tuple(range(n_params, n_params + n_outs))
    sharded = jax.jit(
        shard_map(
            _body,
            mesh=mesh,
            in_specs=(PartitionSpec("core"),) * (n_params + n_outs),
            out_specs=(PartitionSpec("core"),) * n_outs,
            check_rep=False,
        ),
        donate_argnums=donate,
        keep_unused=True,
    )
    sh = NamedSharding(mesh, PartitionSpec("core"))
    _EXEC = (sharded, in_names, out_names, out_avals, sh)
    return _EXEC


def _run(in_maps, trace=False):
    """Execute with inputs pre-staged on the devices so all 8 ranks start
    aligned.  Returns (per-core results, exec_time_ns, profile_json)."""
    import jax

    sharded, in_names, out_names, out_avals, sh = _get_exec()
    nc = _NC
    concat_in = [
        np.concatenate([np.asarray(m[n]) for m in in_maps], axis=0)
        for n in in_names
    ]
    staged = [jax.device_put(a, sh) for a in concat_in]
    zeros = [
        jax.device_put(np.zeros((M * av.shape[0], *av.shape[1:]), av.dtype), sh)
        for av in out_avals
    ]
    jax.block_until_ready(staged)
    jax.block_until_ready(zeros)

    exec_time_ns = None
    profile_json = None
    if trace:
        try:
            from antenv.axon_hooks import get_axon_ntff_profile_hook

            hook = get_axon_ntff_profile_hook()
        except Exception:
            hook = None
        if hook is not None:
            import gauge.profiler

            bass_utils.upload_artifacts = lambda tmpdir: "local://skipped"
            td = tempfile.mkdtemp()
            with hook(td, [0]):
                out_arrs = sharded(*staged, *zeros)
                jax.block_until_ready(out_arrs)
            ntffs = glob.glob(os.path.join(td, "*_body*.ntff"))
            if ntffs:
                profile = gauge.profiler.Profile(
                    profile_path=bass_utils.FishPath(td),
                    kernel_dev_mode=True,
                    profile_on_exit=False,
                    bass_kernel=nc.m,
                    offline_processing=True,
                    fname="*_body*",
                    metadata={"artifacts_path": "local://skipped"},
                )
                perf = bass_utils._process_ntff_profile(
                    profile, td, nc, list(range(M)), None, False, {}, False
                )
                exec_time_ns = perf.exec_time_ns
                profile_json = perf.profile_json
        else:
            out_arrs = sharded(*staged, *zeros)
            jax.block_until_ready(out_arrs)
    else:
        out_arrs = sharded(*staged, *zeros)
        jax.block_until_ready(out_arrs)

    results = [
        {
            name: np.asarray(out_arrs[i]).reshape(M, *out_avals[i].shape)[c]
            for i, name in enumerate(out_names)
        }
        for c in range(M)
    ]
    return results, exec_time_ns, profile_json


def kernel(**inputs) -> np.ndarray:
    in_maps = _shard_inputs(inputs)
    trace = bool(os.environ.get("KERNEL_TRACE"))
    repeat = int(os.environ.get("KERNEL_REPEAT", "1"))
    for _ in range(repeat - 1):
        _run(in_maps, trace=False)
    results, exec_ns, prof = _run(in_maps, trace=trace)
    kernel.last_exec_time_ns = exec_ns
    kernel.last_profile_json = prof
    return np.asarray(np.float32(results[0]["out"][0])).reshape(())


def emulate(**inputs) -> np.ndarray:
    """Numpy emulation of the exact device dataflow (for layout validation)."""
    maps = _shard_inputs(inputs)
    packs = []
    for m in range(M):
        mp = maps[m]
        etp, oh = mp["etp"], mp["oh"]
        pack = np.zeros((C + 1, EB), np.float32)
        for t in range(T):
            lhsT = oh[:, t * (C + 1) : (t + 1) * (C + 1)]  # [128, 11]
            rhs = etp[:, t * EB : (t + 1) * EB]  # [128, EB]
            pack += lhsT.T @ rhs
        packs.append(pack)
    S = np.sum(packs, axis=0)  # AllGather + local reduce
    R = np.maximum(S[:, 0:D], 0.0)  # [11, 300]
    srow = np.zeros((768,), np.float32)
    srow[0:D] = C * R[0]
    srow[384 : 384 + D] = R.sum(axis=0) - R[0]
    scol = srow.reshape(6, 128).T  # [128, 6]
    mp = maps[0]
    us = np.zeros((256,), np.float32)
    for j in range(6):
        us += scol[:, j] @ mp["wms"][:, j * 256 : (j + 1) * 256]

    def sp(x):
        return np.maximum(x, 0) + np.log1p(np.exp(-np.abs(x)))

    u = us[0:Z] + mp["bmu"]
    s = sp(us[Z : 2 * Z] + mp["bsg"])
    z = u + mp["eps"] * s
    zs = sp(S[0, D:EB])
    kl = np.log(zs) - np.log(s) + (s**2 + (u - zs) ** 2) * 0.5 / zs**2 - 0.5
    klsum = kl.sum()

    pairs = []
    for m in range(M):
        mp_ = maps[m]
        lflat = z @ mp_["wgt"]  # [VP]
        logits = lflat.reshape(T, 128).T + mp_["bgt"]
        lmax = logits.max()
        esum = np.exp(logits - lmax).sum()
        pairs.append((lmax, esum))
    gmax = max(p[0] for p in pairs)
    gsum = sum(p[1] * np.exp(p[0] - gmax) for p in pairs)
    cl = z @ mp["wgc"] + mp["bgc"]
    resv = cl.sum() - C * (gmax + np.log(gsum)) - klsum
    return np.asarray(np.float32(resv)).reshape(())
